# revision 27
# baseline (speedup 1.0000x reference)
"""Single-head attention (S=8192, D=1024, d_k=128) on 8 TRN2 NeuronCores.

Strategy: sequence-parallel. Each core owns SL=1024 query rows. Per core:
  - transpose the x shard via PE transposes to get x^T tiles
  - project K^T (dual epilogue: f32r + bf16 copies straight from PSUM),
    V^T (PE-transposed to V natural, stored bf16), then Q^T with SCALE
    folded into the activation epilogue (f32r)
  - three AllGathers, ordered to pipeline the serial collective stream:
    bf16 K^T first (half the bytes -> the DVE-paced max phase starts
    early), then f32r K^T (feeds p2 scores), then bf16 V; gathers are
    issued right after their producers so transfers start as soon as the
    ~60us CC init barrier ends
  - max phase (p1), per 128-query block in natural orientation (bf16):
    Q^T block stationary, K^T moving -> S_nat chunks in PSUM; DVE
    reduce_max over the free dim straight from PSUM; combine partials,
    negate, PE-transpose into a [1, SL] row of -rowmax. p1 is DVE-paced
    (~700ns per 512-wide PSUM reduce), so blocks 0-3 run before group 0
    and blocks 4-7 are injected chunk-by-chunk into group 0's k-loop to
    keep the PE busy while the DVE chews.
  - attention phase (p2), per 512-query group, per 128-k tile:
    PSUM := -m via a K=1 bf16 ones x negmrow matmul (start=True) with the
    NEXT tile's preload emitted right after this tile's exp so its PSUM
    drain hides under other matmuls; S^T matmul (f32r, 1 cyc/row)
    accumulates on top (start=False) -> PSUM holds SCALE*S - m; ACT exps
    PSUM -> SBUF bf16 P^T tile; PV matmuls accumulate O^T across k into
    even/odd PSUM banks (alternating banks overlaps the accumulation
    drain); row sums l accumulate off-PE via gpsimd (3/4) and DVE (1/4)
    running adds over the P^T tiles
  - epilogue per group: l = ones^T @ (laccg + laccv) matmul, transpose,
    DVE reciprocal -> per-q-partition 1/l; O^T halves merged (ACT copy +
    DVE add), PE-transposed, scaled by 1/l in the ACT copy, DMA out.

Matmul dtypes: fp32 operands are produced as float32r (1 cycle/row for
free-dim >= 256 vs 4 for plain fp32; walrus requires producers to round
explicitly); P/V/preload matmuls run in bf16. The row max only needs to
be within ~+-80 of the true max (the softmax shift cancels exactly), so
bf16/f32r rounding on the max path is harmless; exp arguments stay
<= ~+16 so nothing overflows.

Hard-won toolchain constraints baked in here:
  - walrus allows ONE sync wait per Matmult: DMA-fed matmul operands get
    a tiny "absorber" ldweights after their DMA, and split_multi_waits
    hoists any remaining extra waits into EventSemaphore instructions
  - a lone start=False f32r matmul onto engine-written PSUM only
    accumulates its first 128-column row group (hence the matmul preload,
    not an ACT/DVE copy); DVE/ACT writes to PSUM can't replace it
  - GPSIMD cannot access PSUM at all
  - collective triggers block the issuing engine (gpsimd) while a prior
    collective is in flight; gathered-tensor loads stay on gpsimd since
    SP-issued loads showed a sporadic race against collective completion
  - matmul PSUM outputs must fit one 2KB bank -> 512 fp32 free-dim cap
"""

import math
import os
import sys
from contextlib import ExitStack

for _p in ("/opt/trn_rl_repo", os.path.expanduser("~/.axon_site/_ro/trn_rl_repo")):
    if os.path.isdir(_p) and _p not in sys.path:
        sys.path.insert(0, _p)

import numpy as np

import concourse.bass as bass
import concourse.mybir as mybir
import concourse.tile as tile
from concourse.bass_utils import run_bass_kernel_spmd
from concourse.masks import make_identity

S = 8192
D = 1024
DK = 128
NC = 8
SL = S // NC  # 1024 query rows per core
SCALE = 1.0 / math.sqrt(DK)
FP32 = mybir.dt.float32
F32R = mybir.dt.float32r
BF16 = mybir.dt.bfloat16
Act = mybir.ActivationFunctionType
Alu = mybir.AluOpType

NQ = 512          # queries per p2 group
NG = SL // NQ     # 4 groups
NB = SL // 128    # 8 query blocks for p1
NKT = S // 128    # 64 k tiles
KC = 512          # k chunk width for p1 (one PSUM bank)
NCH = S // KC     # 16 p1 chunks per block


def build_program() -> bass.Bass:
    nc = bass.Bass(num_devices=NC)

    x_sh = nc.declare_dram_parameter("x_sh", [SL, D], FP32, isOutput=False)
    w_q = nc.declare_dram_parameter("W_Q", [D, DK], FP32, isOutput=False)
    b_q = nc.declare_dram_parameter("b_Q", [1, DK], FP32, isOutput=False)
    w_k = nc.declare_dram_parameter("W_K", [D, DK], FP32, isOutput=False)
    b_k = nc.declare_dram_parameter("b_K", [1, DK], FP32, isOutput=False)
    w_v = nc.declare_dram_parameter("W_V", [D, DK], FP32, isOutput=False)
    b_v = nc.declare_dram_parameter("b_V", [1, DK], FP32, isOutput=False)
    out_sh = nc.declare_dram_parameter("out_sh", [SL, DK], FP32, isOutput=True)

    groups = [list(range(NC))]

    with tile.TileContext(nc) as tc, ExitStack() as ctx:
        dram = ctx.enter_context(tc.tile_pool(name="dram", bufs=1, space="DRAM"))
        khl_d = dram.tile([DK, SL], BF16)
        khg_d = dram.tile([NC * DK, SL], BF16, addr_space="Shared")
        ktl_d = dram.tile([DK, SL], F32R)
        ktg_d = dram.tile([NC * DK, SL], F32R, addr_space="Shared")
        vnl_d = dram.tile([SL, DK], BF16)
        vng_d = dram.tile([S, DK], BF16, addr_space="Shared")
        const = ctx.enter_context(tc.tile_pool(name="const", bufs=1))
        big = ctx.enter_context(tc.tile_pool(name="big", bufs=1))
        stat = ctx.enter_context(tc.tile_pool(name="stat", bufs=2))
        work = ctx.enter_context(tc.tile_pool(name="work", bufs=4))
        outp = ctx.enter_context(tc.tile_pool(name="outp", bufs=3))
        # PSUM budget (8 banks): psP1 2x[128,512] (also hosts proj matmuls)
        #   + psS 3x[128,256] (scores, 128x128 transposes) + psO 1 + psL 1
        #   + pssm 1 (tiny stat transposes)
        psP1 = ctx.enter_context(tc.tile_pool(name="psP1", bufs=2, space="PSUM"))
        psS = ctx.enter_context(tc.tile_pool(name="psS", bufs=3, space="PSUM"))
        psO = ctx.enter_context(tc.tile_pool(name="psO", bufs=1, space="PSUM"))
        pssm = ctx.enter_context(tc.tile_pool(name="pssm", bufs=1, space="PSUM"))

        def absorb(col_ap):
            """1-wait PE ldweights folding col_ap's producer sem into PE's clock.

            Bare InstLdweights has no output, so it builds no WAW chain; the
            bf16 bitcast sidesteps the fp32 standalone-ldweights restriction
            (the loaded garbage weights are never used -- every real matmul
            self-loads since ldw-opt is disabled).
            """
            if os.environ.get("LDW_OPT", "0") != "1":
                nc.tensor.ldweights(weights=col_ap.bitcast(BF16))

        ident = const.tile([128, 128], FP32)
        make_identity(nc, ident[:, :])
        absorb(ident[:, 0:1])
        ones_rf = const.tile([1, 128], FP32, tag="ones_rf")
        nc.gpsimd.memset(ones_rf[:, :], 1.0)
        ones_row = const.tile([1, 128], F32R, tag="ones_row")
        nc.scalar.copy(ones_row[0:1, :], ones_rf[0:1, :])
        absorb(ones_row[0:1, 0:1])
        identb = const.tile([128, 128], BF16, tag="identb")
        nc.scalar.copy(identb[:, :], ident[:, :])
        absorb(identb[:, 0:1])
        ones_rb = const.tile([1, 128], BF16, tag="ones_rb")
        nc.scalar.copy(ones_rb[0:1, :], ones_rf[0:1, :])
        absorb(ones_rb[0:1, 0:1])
        ones_cf = const.tile([128, 1], FP32, tag="ones_cf")
        nc.gpsimd.memset(ones_cf[:, :], 1.0)
        ones_cr = const.tile([128, 1], F32R, tag="ones_cr")
        nc.scalar.copy(ones_cr[:, 0:1], ones_cf[:, 0:1])
        absorb(ones_cr[:, 0:1])

        def load_bias_T(b_dram, tag):
            t = const.tile([128, 1], FP32, tag=tag)
            nc.sync.dma_start(out=t[:, 0], in_=b_dram[0, :])
            return t

        bqT = load_bias_T(b_q, "bqT")
        bkT = load_bias_T(b_k, "bkT")
        bvT = load_bias_T(b_v, "bvT")
        # Q epilogue computes SCALE*ps + (SCALE*b)
        bqTs = const.tile([128, 1], FP32, tag="bqTs")
        nc.scalar.mul(bqTs[:, :], bqT[:, :], SCALE)

        proj_ctx = ExitStack()
        xpool = proj_ctx.enter_context(tc.tile_pool(name="xpool", bufs=1))
        xload = proj_ctx.enter_context(tc.tile_pool(name="xload", bufs=8))
        wpool = proj_ctx.enter_context(tc.tile_pool(name="wpool", bufs=3))

        def load_w(w_dram, tag):
            wt = wpool.tile([128, D // 128, DK], FP32, tag="wt")
            nc.gpsimd.dma_start(out=wt[:, :, :], in_=w_dram.rearrange("(t p) k -> p t k", p=128))
            absorb(wt[:, 0, 0:1])
            wtr = wpool.tile([128, D // 128, DK], F32R, tag="wtr")
            nc.scalar.copy(wtr[:, :, :], wt[:, :, :])
            return wtr

        wK = load_w(w_k, "wK")
        wV = load_w(w_v, "wV")
        wQ = load_w(w_q, "wQ")

        # x^T: [128 d-part, 8 d-tiles, SL seq]  (d = dt*128 + partition)
        xT = xpool.tile([128, D // 128, SL], F32R)

        def xt_half(h):
            for st in range(h * 4, h * 4 + 4):
                xn = xload.tile([128, D], FP32)
                nc.gpsimd.dma_start(out=xn[:, :], in_=x_sh[st * 128 : (st + 1) * 128, :])
                absorb(xn[:, 0:1])
                for dt in range(D // 128):
                    pt = psS.tile([128, 128], FP32, tag="ps")
                    nc.tensor.transpose(pt[:, :], xn[:, dt * 128 : (dt + 1) * 128], ident[:, :])
                    if dt % 2 == 0:
                        nc.scalar.copy(xT[:, dt, st * 128 : (st + 1) * 128], pt[:, :])
                    else:
                        nc.vector.tensor_copy(xT[:, dt, st * 128 : (st + 1) * 128], pt[:, :])

        def proj_half(wtr, outT, bT, h, scale=1.0, outB=None):
            """outT[:, h*512:(h+1)*512] = (scale*(x_h @ W) + scale*b)^T;
            outB (optional, emitted FIRST) gets a bf16 copy of the same."""
            ps = psP1.tile([128, 512], FP32, tag="p1")
            for dt in range(D // 128):
                nc.tensor.matmul(
                    ps[:, :],
                    lhsT=wtr[:, dt, :],
                    rhs=xT[:, dt, h * 512 : (h + 1) * 512],
                    start=(dt == 0),
                    stop=(dt == D // 128 - 1),
                )
            if outB is not None:
                nc.scalar.activation(outB[:, h * 512 : (h + 1) * 512], ps[:, :], Act.Identity, bias=bT[:, :], scale=scale)
            nc.scalar.activation(outT[:, h * 512 : (h + 1) * 512], ps[:, :], Act.Identity, bias=bT[:, :], scale=scale)

        # K first; a bf16 copy of K^T gathers FIRST (half the bytes) so the
        # DVE-paced max phase can start before the f32r K and V land
        ktl = big.tile([128, SL], F32R)
        khl = big.tile([128, SL], BF16, tag="khl")
        xt_half(0)
        proj_half(wK, ktl, bkT, 0, outB=khl)
        xt_half(1)
        proj_half(wK, ktl, bkT, 1, outB=khl)
        nc.gpsimd.dma_start(out=khl_d[:, :], in_=khl[:, :])
        nc.gpsimd.collective_compute(
            "AllGather", Alu.bypass, replica_groups=groups, ins=[khl_d[:, :]], outs=[khg_d[:, :]]
        )
        nc.gpsimd.dma_start(out=ktl_d[:, :], in_=ktl[:, :])
        nc.gpsimd.collective_compute(
            "AllGather", Alu.bypass, replica_groups=groups, ins=[ktl_d[:, :]], outs=[ktg_d[:, :]]
        )

        vtl = big.tile([128, SL], F32R)
        proj_half(wV, vtl, bvT, 0)
        proj_half(wV, vtl, bvT, 1)
        vnl = big.tile([128, SL // 128, DK], BF16)
        for st in range(SL // 128):
            pt = psS.tile([128, 128], FP32, tag="ps")
            nc.tensor.transpose(pt[:, :], vtl.bitcast(FP32)[:, st * 128 : (st + 1) * 128], ident[:, :])
            nc.scalar.copy(vnl[:, st, :], pt[:, :])
        nc.gpsimd.dma_start(out=vnl_d.rearrange("(t p) k -> p t k", p=128), in_=vnl[:, :, :])
        nc.gpsimd.collective_compute(
            "AllGather", Alu.bypass, replica_groups=groups, ins=[vnl_d[:, :]], outs=[vng_d[:, :]]
        )

        # Q (not gather-dependent); pre-scaled by SCALE
        qT = big.tile([128, SL], F32R)
        proj_half(wQ, qT, bqTs, 0, scale=SCALE)
        proj_half(wQ, qT, bqTs, 1, scale=SCALE)
        proj_ctx.close()

        # gathered K^T [128 dk, 8192 k] (fp32) and V natural (bf16)
        qTb = big.tile([128, SL], BF16, tag="qTb")
        nc.scalar.copy(qTb[:, :], qT[:, :])
        ktFb = big.tile([128, NC, SL], BF16, tag="ktFb")
        nc.gpsimd.dma_start(out=ktFb[:, :, :], in_=khg_d.rearrange("(c p) s -> p c s", p=128))
        absorb(ktFb[:, 0, 0:1])
        ktFb2 = ktFb.rearrange("p c s -> p (c s)")
        ktF = big.tile([128, NC, SL], F32R)
        nc.gpsimd.dma_start(out=ktF[:, :, :], in_=ktg_d.rearrange("(c p) s -> p c s", p=128))
        absorb(ktF[:, 0, 0:1])
        ktF2 = ktF.rearrange("p c s -> p (c s)")
        vnF = big.tile([128, S // 128, DK], BF16)
        nc.gpsimd.dma_start(out=vnF[:, :, :], in_=vng_d.rearrange("(t p) k -> p t k", p=128))
        absorb(vnF[:, 0, 0:1])

        # -rowmax as a [1, SL] row, built per 128-q block by p1
        negmrow = stat.tile([1, SL], BF16, tag="negmrow", bufs=1)

        def p1_parts(b):
            """p1 for block b as (A_chunks, B_chunks, finalize) closures."""
            pmax = stat.tile([128, NCH], FP32, tag="pmax", bufs=4)

            def chunk(c):
                def emit():
                    ps = psP1.tile([128, KC], FP32, tag="p1")
                    nc.tensor.matmul(
                        ps[:, :],
                        lhsT=qTb[:, b * 128 : (b + 1) * 128],
                        rhs=ktFb2[:, c * KC : (c + 1) * KC],
                        start=True, stop=True,
                    )
                    nc.vector.reduce_max(pmax[:, c : c + 1], ps[:, :], axis=mybir.AxisListType.X)
                return emit

            def finalize():
                negm = stat.tile([128, 1], FP32, tag="negm")
                nc.vector.reduce_max(negm[:, :], pmax[:, :], axis=mybir.AxisListType.X, negate=True)
                pt = pssm.tile([1, 128], FP32, tag="sm")
                nc.tensor.transpose(pt[0:1, :], negm[:, :], ident[:, :])
                nc.vector.tensor_copy(negmrow[0:1, b * 128 : (b + 1) * 128], pt[0:1, :])

            A = [chunk(c) for c in range(0, NCH, 2)]
            B = [chunk(c) for c in range(1, NCH, 2)]
            return A, B, finalize

        def p1_closures(b):
            A, B, fin = p1_parts(b)
            return A + B + [fin]

        def p2_group(g, inject=()):
            """exp + PV + running l for query group g; `inject` closures are
            drained one per k-iteration (p1 work for later blocks)."""
            inject = list(inject)
            qs = slice(g * NQ, (g + 1) * NQ)
            pso_e = psO.tile([128, NQ], FP32, tag="oe")
            pso_o = psO.tile([128, NQ], FP32, tag="oo")
            pso = [pso_e, pso_o]
            laccg = stat.tile([128, NQ], FP32, tag="laccg", bufs=1)
            laccv = stat.tile([128, NQ], FP32, tag="laccv", bufs=1)
            pts = [None] * NKT
            pss = [None] * NKT

            # broadcast -m across partitions once per group (bf16)
            psB = psS.tile([128, NQ], FP32, tag="ps")
            nc.tensor.matmul(psB[:, :], lhsT=ones_rb[0:1, :], rhs=negmrow[0:1, qs],
                             start=True, stop=True)
            negmbc = stat.tile([128, NQ], BF16, tag="negmbc", bufs=1)
            nc.scalar.copy(negmbc[:, :], psB[:, :])

            def preload(kt):
                ps = psS.tile([128, NQ], FP32, tag="ps")
                nc.tensor.matmul(
                    ps[:, :], lhsT=identb[:, :], rhs=negmbc[:, :],
                    start=True, stop=False, skip_group_check=True,
                )
                pss[kt] = ps

            def pv(kt):
                nc.tensor.matmul(
                    pso[kt % 2][:, :], lhsT=vnF[:, kt, :], rhs=pts[kt][:, :],
                    start=(kt < 2), stop=(kt >= NKT - 2), skip_group_check=True,
                )

            preload(0)
            for kt in range(NKT):
                nc.tensor.matmul(
                    pss[kt][:, :], lhsT=ktF2[:, kt * 128 : (kt + 1) * 128], rhs=qT[:, qs],
                    start=False, stop=True, skip_group_check=True,
                )
                pt = work.tile([128, NQ], BF16, tag="ptile")
                nc.scalar.activation(pt[:, :], pss[kt][:, :], Act.Exp)
                pts[kt] = pt
                # next preload right away: its PSUM-write drain hides under
                # the PV/inject matmuls instead of stalling the next score
                if kt + 1 < NKT:
                    preload(kt + 1)
                # running row-sum contributions off the PE: 3/4 gpsimd, 1/4 DVE
                if kt == 0:
                    nc.gpsimd.tensor_copy(laccg[:, :], pt[:, :])
                elif kt == 3:
                    nc.vector.tensor_copy(laccv[:, :], pt[:, :])
                elif kt % 4 == 3:
                    nc.vector.tensor_add(laccv[:, :], laccv[:, :], pt[:, :])
                else:
                    nc.gpsimd.tensor_add(laccg[:, :], laccg[:, :], pt[:, :])
                if inject:
                    inject.pop(0)()
                # PV for kt-2 keeps PE two tiles behind ACT (no stall)
                if kt >= 2:
                    pv(kt - 2)
            for kt in (NKT - 2, NKT - 1):
                pv(kt)
            while inject:
                inject.pop(0)()

            # l = colsum(laccg + laccv) via ones matmul; then 1/l per q
            laccs = stat.tile([128, NQ], F32R, tag="laccs")
            nc.vector.tensor_add(laccs[:, :], laccg[:, :], laccv[:, :])
            psl = pssm.tile([1, NQ], FP32, tag="sm")
            nc.tensor.matmul(psl[0:1, :], lhsT=ones_cr[:, :], rhs=laccs[:, :],
                             start=True, stop=True)
            lrow = stat.tile([1, NQ], FP32, tag="lrow")
            nc.scalar.copy(lrow[0:1, :], psl[0:1, :])
            otn = work.tile([128, NQ], FP32, tag="otn")
            nc.scalar.copy(otn[:, :], pso[0][:, :])
            nc.vector.tensor_add(otn[:, :], otn[:, :], pso[1][:, :])
            for qt in range(NQ // 128):
                plt = pssm.tile([128, 1], FP32, tag="sm")
                nc.tensor.transpose(plt[:, 0:1], lrow[0:1, qt * 128 : (qt + 1) * 128], ident[0:1, 0:1])
                linv = stat.tile([128, 1], FP32, tag="linv")
                nc.vector.reciprocal(linv[:, :], plt[:, 0:1])
                po = psS.tile([128, 128], FP32, tag="ps")
                nc.tensor.transpose(po[:, :], otn[:, qt * 128 : (qt + 1) * 128], ident[:, :])
                ot = outp.tile([128, 128], FP32)
                nc.scalar.activation(ot[:, :], po[:, :], Act.Identity, scale=linv[:, :])
                q0 = g * NQ + qt * 128
                nc.gpsimd.dma_start(out=out_sh[q0 : q0 + 128, :], in_=ot[:, :])

        # p1 blocks 0-3 up front: all half-A chunks first (their gather
        # lands earlier), then half-B + finalize; blocks 4-7 inject into g0
        parts03 = [p1_parts(b) for b in range(4)]
        for A, _, _ in parts03:
            for cl in A:
                cl()
        for _, B, fin in parts03:
            for cl in B:
                cl()
            fin()
        p2_group(0, inject=p1_closures(4) + p1_closures(5) + p1_closures(6) + p1_closures(7))
        p2_group(1)

    split_multi_waits(nc)
    return nc


def split_multi_waits(nc):
    """Hoist all-but-one sync wait off engine/DMA instructions into
    standalone EventSemaphore instructions.

    This toolchain's walrus build has a single wait-command slot per
    non-sequencer instruction; Tile emits multi-wait instructions assuming
    a newer codegen. Sequencer sync instructions (EventSemaphore, Drain)
    accept arbitrary waits, so semantics are preserved by hoisting.
    """
    import bass_rust

    exempt = {"InstEventSemaphore"}
    n_split = 0
    for f in nc.m.functions:
        for bb in f.blocks:
            out = []
            changed = False
            for ins in bb.instructions:
                si = ins.sync_info
                if (
                    si is not None
                    and len(si.on_wait) > 1
                    and type(ins).__name__ not in exempt
                    and ins.engine is not None
                ):
                    for j, w in enumerate(si.on_wait[:-1]):
                        ev = mybir.InstEventSemaphore(
                            name=f"{ins.name}-wsplit{j}", ins=[], outs=[]
                        )
                        ev.engine = ins.engine
                        ev.sync_info = bass_rust.SyncInfo(on_wait=[w], on_update=[])
                        out.append(ev)
                        n_split += 1
                    ins.sync_info = bass_rust.SyncInfo(
                        on_wait=[si.on_wait[-1]], on_update=list(si.on_update)
                    )
                    changed = True
                out.append(ins)
            if changed:
                bb.instructions = out
    return n_split


_PROGRAM = None


def _get_program():
    global _PROGRAM
    if _PROGRAM is None:
        _PROGRAM = build_program()
    return _PROGRAM


def kernel(x, W_Q, b_Q, W_K, b_K, W_V, b_V):
    x = np.ascontiguousarray(np.asarray(x, dtype=np.float32))
    args = {
        "W_Q": np.ascontiguousarray(np.asarray(W_Q, dtype=np.float32)),
        "b_Q": np.ascontiguousarray(np.asarray(b_Q, dtype=np.float32)),
        "W_K": np.ascontiguousarray(np.asarray(W_K, dtype=np.float32)),
        "b_K": np.ascontiguousarray(np.asarray(b_K, dtype=np.float32)),
        "W_V": np.ascontiguousarray(np.asarray(W_V, dtype=np.float32)),
        "b_V": np.ascontiguousarray(np.asarray(b_V, dtype=np.float32)),
    }
    nc = _get_program()
    in_maps = [dict(args, x_sh=x[c * SL : (c + 1) * SL]) for c in range(NC)]
    res = run_bass_kernel_spmd(nc, in_maps, list(range(NC)))
    return np.concatenate([res.results[c]["out_sh"] for c in range(NC)], axis=0)


# revision 28
# speedup vs baseline: 1.0566x; 1.0566x over previous
"""Single-head attention (S=8192, D=1024, d_k=128) on 8 TRN2 NeuronCores.

Strategy: sequence-parallel. Each core owns SL=1024 query rows. Per core:
  - transpose the x shard via PE transposes to get x^T tiles
  - project K^T (dual epilogue: f32r + bf16 copies straight from PSUM),
    V^T (PE-transposed to V natural, stored bf16), then Q^T with SCALE
    folded into the activation epilogue (f32r)
  - three AllGathers, ordered to pipeline the serial collective stream:
    bf16 K^T first (half the bytes -> the DVE-paced max phase starts
    early), then f32r K^T (feeds p2 scores), then bf16 V; gathers are
    issued right after their producers so transfers start as soon as the
    ~60us CC init barrier ends
  - max phase (p1), per 128-query block in natural orientation (bf16):
    Q^T block stationary, K^T moving -> S_nat chunks in PSUM; DVE
    reduce_max over the free dim straight from PSUM; combine partials,
    negate, PE-transpose into a [1, SL] row of -rowmax. p1 is DVE-paced
    (~700ns per 512-wide PSUM reduce), so blocks 0-3 run before group 0
    and blocks 4-7 are injected chunk-by-chunk into group 0's k-loop to
    keep the PE busy while the DVE chews.
  - attention phase (p2), per 512-query group, per 128-k tile:
    PSUM := -m via a K=1 bf16 ones x negmrow matmul (start=True) with the
    NEXT tile's preload emitted right after this tile's exp so its PSUM
    drain hides under other matmuls; S^T matmul (f32r, 1 cyc/row)
    accumulates on top (start=False) -> PSUM holds SCALE*S - m; ACT exps
    PSUM -> SBUF bf16 P^T tile; PV matmuls accumulate O^T across k into
    even/odd PSUM banks (alternating banks overlaps the accumulation
    drain); row sums l accumulate off-PE via gpsimd (3/4) and DVE (1/4)
    running adds over the P^T tiles
  - epilogue per group: l = ones^T @ (laccg + laccv) matmul, transpose,
    DVE reciprocal -> per-q-partition 1/l; O^T halves merged (ACT copy +
    DVE add), PE-transposed, scaled by 1/l in the ACT copy, DMA out.

Matmul dtypes: fp32 operands are produced as float32r (1 cycle/row for
free-dim >= 256 vs 4 for plain fp32; walrus requires producers to round
explicitly); P/V/preload matmuls run in bf16. The row max only needs to
be within ~+-80 of the true max (the softmax shift cancels exactly), so
bf16/f32r rounding on the max path is harmless; exp arguments stay
<= ~+16 so nothing overflows.

Hard-won toolchain constraints baked in here:
  - walrus allows ONE sync wait per Matmult: DMA-fed matmul operands get
    a tiny "absorber" ldweights after their DMA, and split_multi_waits
    hoists any remaining extra waits into EventSemaphore instructions
  - a lone start=False f32r matmul onto engine-written PSUM only
    accumulates its first 128-column row group (hence the matmul preload,
    not an ACT/DVE copy); DVE/ACT writes to PSUM can't replace it
  - GPSIMD cannot access PSUM at all
  - collective triggers block the issuing engine (gpsimd) while a prior
    collective is in flight; gathered-tensor loads stay on gpsimd since
    SP-issued loads showed a sporadic race against collective completion
  - matmul PSUM outputs must fit one 2KB bank -> 512 fp32 free-dim cap
"""

import math
import os
import sys
from contextlib import ExitStack

for _p in ("/opt/trn_rl_repo", os.path.expanduser("~/.axon_site/_ro/trn_rl_repo")):
    if os.path.isdir(_p) and _p not in sys.path:
        sys.path.insert(0, _p)

import numpy as np

import concourse.bass as bass
import concourse.mybir as mybir
import concourse.tile as tile
from concourse.bass_utils import run_bass_kernel_spmd
from concourse.masks import make_identity

S = 8192
D = 1024
DK = 128
NC = 8
SL = S // NC  # 1024 query rows per core
SCALE = 1.0 / math.sqrt(DK)
FP32 = mybir.dt.float32
F32R = mybir.dt.float32r
BF16 = mybir.dt.bfloat16
Act = mybir.ActivationFunctionType
Alu = mybir.AluOpType

NQ = 512          # queries per p2 group
NG = SL // NQ     # 4 groups
NB = SL // 128    # 8 query blocks for p1
NKT = S // 128    # 64 k tiles
KC = 512          # k chunk width for p1 (one PSUM bank)
NCH = S // KC     # 16 p1 chunks per block


def build_program() -> bass.Bass:
    nc = bass.Bass(num_devices=NC)

    x_sh = nc.declare_dram_parameter("x_sh", [SL, D], FP32, isOutput=False)
    w_q = nc.declare_dram_parameter("W_Q", [D, DK], FP32, isOutput=False)
    b_q = nc.declare_dram_parameter("b_Q", [1, DK], FP32, isOutput=False)
    w_k = nc.declare_dram_parameter("W_K", [D, DK], FP32, isOutput=False)
    b_k = nc.declare_dram_parameter("b_K", [1, DK], FP32, isOutput=False)
    w_v = nc.declare_dram_parameter("W_V", [D, DK], FP32, isOutput=False)
    b_v = nc.declare_dram_parameter("b_V", [1, DK], FP32, isOutput=False)
    out_sh = nc.declare_dram_parameter("out_sh", [SL, DK], FP32, isOutput=True)

    groups = [list(range(NC))]

    with tile.TileContext(nc) as tc, ExitStack() as ctx:
        dram = ctx.enter_context(tc.tile_pool(name="dram", bufs=1, space="DRAM"))
        khl_d = dram.tile([DK, SL], BF16)
        khg_d = dram.tile([NC * DK, SL], BF16, addr_space="Shared")
        ktl_d = dram.tile([DK, SL], F32R)
        ktg_d = dram.tile([NC * DK, SL], F32R, addr_space="Shared")
        vnl_d = dram.tile([SL, DK], BF16)
        vng_d = dram.tile([S, DK], BF16, addr_space="Shared")
        const = ctx.enter_context(tc.tile_pool(name="const", bufs=1))
        big = ctx.enter_context(tc.tile_pool(name="big", bufs=1))
        stat = ctx.enter_context(tc.tile_pool(name="stat", bufs=2))
        work = ctx.enter_context(tc.tile_pool(name="work", bufs=4))
        outp = ctx.enter_context(tc.tile_pool(name="outp", bufs=3))
        # PSUM budget (8 banks): psP1 2x[128,512] (also hosts proj matmuls)
        #   + psS 3x[128,256] (scores, 128x128 transposes) + psO 1 + psL 1
        #   + pssm 1 (tiny stat transposes)
        psP1 = ctx.enter_context(tc.tile_pool(name="psP1", bufs=2, space="PSUM"))
        psS = ctx.enter_context(tc.tile_pool(name="psS", bufs=3, space="PSUM"))
        psO = ctx.enter_context(tc.tile_pool(name="psO", bufs=1, space="PSUM"))
        pssm = ctx.enter_context(tc.tile_pool(name="pssm", bufs=1, space="PSUM"))

        def absorb(col_ap):
            """1-wait PE ldweights folding col_ap's producer sem into PE's clock.

            Bare InstLdweights has no output, so it builds no WAW chain; the
            bf16 bitcast sidesteps the fp32 standalone-ldweights restriction
            (the loaded garbage weights are never used -- every real matmul
            self-loads since ldw-opt is disabled).
            """
            if os.environ.get("LDW_OPT", "0") != "1":
                nc.tensor.ldweights(weights=col_ap.bitcast(BF16))

        ident = const.tile([128, 128], FP32)
        make_identity(nc, ident[:, :])
        absorb(ident[:, 0:1])
        ones_rf = const.tile([1, 128], FP32, tag="ones_rf")
        nc.gpsimd.memset(ones_rf[:, :], 1.0)
        ones_row = const.tile([1, 128], F32R, tag="ones_row")
        nc.scalar.copy(ones_row[0:1, :], ones_rf[0:1, :])
        absorb(ones_row[0:1, 0:1])
        ones_rb = const.tile([1, 128], BF16, tag="ones_rb")
        nc.scalar.copy(ones_rb[0:1, :], ones_rf[0:1, :])
        absorb(ones_rb[0:1, 0:1])
        ones_cf = const.tile([128, 1], FP32, tag="ones_cf")
        nc.gpsimd.memset(ones_cf[:, :], 1.0)
        ones_cr = const.tile([128, 1], F32R, tag="ones_cr")
        nc.scalar.copy(ones_cr[:, 0:1], ones_cf[:, 0:1])
        absorb(ones_cr[:, 0:1])

        def load_bias_T(b_dram, tag):
            t = const.tile([128, 1], FP32, tag=tag)
            nc.sync.dma_start(out=t[:, 0], in_=b_dram[0, :])
            return t

        bqT = load_bias_T(b_q, "bqT")
        bkT = load_bias_T(b_k, "bkT")
        bvT = load_bias_T(b_v, "bvT")
        # Q epilogue computes SCALE*ps + (SCALE*b)
        bqTs = const.tile([128, 1], FP32, tag="bqTs")
        nc.scalar.mul(bqTs[:, :], bqT[:, :], SCALE)

        proj_ctx = ExitStack()
        xpool = proj_ctx.enter_context(tc.tile_pool(name="xpool", bufs=1))
        xload = proj_ctx.enter_context(tc.tile_pool(name="xload", bufs=8))
        wpool = proj_ctx.enter_context(tc.tile_pool(name="wpool", bufs=3))

        def load_w(w_dram, tag):
            wt = wpool.tile([128, D // 128, DK], FP32, tag="wt")
            nc.gpsimd.dma_start(out=wt[:, :, :], in_=w_dram.rearrange("(t p) k -> p t k", p=128))
            absorb(wt[:, 0, 0:1])
            wtr = wpool.tile([128, D // 128, DK], F32R, tag="wtr")
            nc.scalar.copy(wtr[:, :, :], wt[:, :, :])
            return wtr

        wK = load_w(w_k, "wK")
        wV = load_w(w_v, "wV")
        wQ = load_w(w_q, "wQ")

        # x^T: [128 d-part, 8 d-tiles, SL seq]  (d = dt*128 + partition)
        xT = xpool.tile([128, D // 128, SL], F32R)

        def xt_half(h):
            for st in range(h * 4, h * 4 + 4):
                xn = xload.tile([128, D], FP32)
                nc.gpsimd.dma_start(out=xn[:, :], in_=x_sh[st * 128 : (st + 1) * 128, :])
                absorb(xn[:, 0:1])
                for dt in range(D // 128):
                    pt = psS.tile([128, 128], FP32, tag="ps")
                    nc.tensor.transpose(pt[:, :], xn[:, dt * 128 : (dt + 1) * 128], ident[:, :])
                    if dt % 2 == 0:
                        nc.scalar.copy(xT[:, dt, st * 128 : (st + 1) * 128], pt[:, :])
                    else:
                        nc.vector.tensor_copy(xT[:, dt, st * 128 : (st + 1) * 128], pt[:, :])

        def proj_half(wtr, outT, bT, h, scale=1.0, outB=None):
            """outT[:, h*512:(h+1)*512] = (scale*(x_h @ W) + scale*b)^T;
            outB (optional, emitted FIRST) gets a bf16 copy of the same."""
            ps = psP1.tile([128, 512], FP32, tag="p1")
            for dt in range(D // 128):
                nc.tensor.matmul(
                    ps[:, :],
                    lhsT=wtr[:, dt, :],
                    rhs=xT[:, dt, h * 512 : (h + 1) * 512],
                    start=(dt == 0),
                    stop=(dt == D // 128 - 1),
                )
            if outB is not None:
                nc.scalar.activation(outB[:, h * 512 : (h + 1) * 512], ps[:, :], Act.Identity, bias=bT[:, :], scale=scale)
            nc.scalar.activation(outT[:, h * 512 : (h + 1) * 512], ps[:, :], Act.Identity, bias=bT[:, :], scale=scale)

        # K first; a bf16 copy of K^T gathers FIRST (half the bytes) so the
        # DVE-paced max phase can start before the f32r K and V land
        ktl = big.tile([128, SL], F32R)
        khl = big.tile([128, SL], BF16, tag="khl")
        xt_half(0)
        proj_half(wK, ktl, bkT, 0, outB=khl)
        xt_half(1)
        proj_half(wK, ktl, bkT, 1, outB=khl)
        nc.gpsimd.dma_start(out=khl_d[:, :], in_=khl[:, :])
        nc.gpsimd.collective_compute(
            "AllGather", Alu.bypass, replica_groups=groups, ins=[khl_d[:, :]], outs=[khg_d[:, :]]
        )
        nc.gpsimd.dma_start(out=ktl_d[:, :], in_=ktl[:, :])
        nc.gpsimd.collective_compute(
            "AllGather", Alu.bypass, replica_groups=groups, ins=[ktl_d[:, :]], outs=[ktg_d[:, :]]
        )

        vtl = big.tile([128, SL], F32R)
        proj_half(wV, vtl, bvT, 0)
        proj_half(wV, vtl, bvT, 1)
        vnl = big.tile([128, SL // 128, DK], BF16)
        for st in range(SL // 128):
            pt = psS.tile([128, 128], FP32, tag="ps")
            nc.tensor.transpose(pt[:, :], vtl.bitcast(FP32)[:, st * 128 : (st + 1) * 128], ident[:, :])
            nc.scalar.copy(vnl[:, st, :], pt[:, :])
        nc.gpsimd.dma_start(out=vnl_d.rearrange("(t p) k -> p t k", p=128), in_=vnl[:, :, :])
        nc.gpsimd.collective_compute(
            "AllGather", Alu.bypass, replica_groups=groups, ins=[vnl_d[:, :]], outs=[vng_d[:, :]]
        )

        # Q (not gather-dependent); pre-scaled by SCALE
        qT = big.tile([128, SL], F32R)
        proj_half(wQ, qT, bqTs, 0, scale=SCALE)
        proj_half(wQ, qT, bqTs, 1, scale=SCALE)
        proj_ctx.close()

        # gathered K^T [128 dk, 8192 k] (fp32) and V natural (bf16)
        qTb = big.tile([128, SL], BF16, tag="qTb")
        nc.scalar.copy(qTb[:, :], qT[:, :])
        ktFb = big.tile([128, NC, SL], BF16, tag="ktFb")
        nc.gpsimd.dma_start(out=ktFb[:, :, :], in_=khg_d.rearrange("(c p) s -> p c s", p=128))
        absorb(ktFb[:, 0, 0:1])
        ktFb2 = ktFb.rearrange("p c s -> p (c s)")
        ktF = big.tile([128, NC, SL], F32R)
        nc.gpsimd.dma_start(out=ktF[:, :, :], in_=ktg_d.rearrange("(c p) s -> p c s", p=128))
        absorb(ktF[:, 0, 0:1])
        ktF2 = ktF.rearrange("p c s -> p (c s)")
        vnF = big.tile([128, S // 128, DK], BF16)
        nc.gpsimd.dma_start(out=vnF[:, :, :], in_=vng_d.rearrange("(t p) k -> p t k", p=128))
        absorb(vnF[:, 0, 0:1])

        # -rowmax as a [1, SL] row, built per 128-q block by p1
        negmrow = stat.tile([1, SL], BF16, tag="negmrow", bufs=1)

        def p1_parts(b):
            """p1 for block b as (A_chunks, B_chunks, finalize) closures."""
            pmax = stat.tile([128, NCH], FP32, tag="pmax", bufs=4)

            def chunk(c):
                def emit():
                    ps = psP1.tile([128, KC], FP32, tag="p1")
                    nc.tensor.matmul(
                        ps[:, :],
                        lhsT=qTb[:, b * 128 : (b + 1) * 128],
                        rhs=ktFb2[:, c * KC : (c + 1) * KC],
                        start=True, stop=True,
                    )
                    nc.vector.reduce_max(pmax[:, c : c + 1], ps[:, :], axis=mybir.AxisListType.X)
                return emit

            def finalize():
                negm = stat.tile([128, 1], FP32, tag="negm")
                nc.vector.reduce_max(negm[:, :], pmax[:, :], axis=mybir.AxisListType.X, negate=True)
                pt = pssm.tile([1, 128], FP32, tag="sm")
                nc.tensor.transpose(pt[0:1, :], negm[:, :], ident[:, :])
                nc.vector.tensor_copy(negmrow[0:1, b * 128 : (b + 1) * 128], pt[0:1, :])

            A = [chunk(c) for c in range(0, NCH, 2)]
            B = [chunk(c) for c in range(1, NCH, 2)]
            return A, B, finalize

        def p1_closures(b):
            A, B, fin = p1_parts(b)
            return A + B + [fin]

        def p2_group(g, inject=()):
            """exp + PV + running l for query group g; `inject` closures are
            drained one per k-iteration (p1 work for later blocks)."""
            inject = list(inject)
            qs = slice(g * NQ, (g + 1) * NQ)
            pso_e = psO.tile([128, NQ], FP32, tag="oe")
            pso_o = psO.tile([128, NQ], FP32, tag="oo")
            pso = [pso_e, pso_o]
            laccg = stat.tile([128, NQ], FP32, tag="laccg", bufs=1)
            laccv = stat.tile([128, NQ], FP32, tag="laccv", bufs=1)
            pts = [None] * NKT
            pss = [None] * NKT

            def preload(kt):
                ps = psS.tile([128, NQ], FP32, tag="ps")
                nc.tensor.matmul(
                    ps[:, :], lhsT=ones_rb[0:1, :], rhs=negmrow[0:1, qs],
                    start=True, stop=False, skip_group_check=True,
                )
                pss[kt] = ps

            def pv(kt):
                nc.tensor.matmul(
                    pso[kt % 2][:, :], lhsT=vnF[:, kt, :], rhs=pts[kt][:, :],
                    start=(kt < 2), stop=(kt >= NKT - 2), skip_group_check=True,
                )

            preload(0)
            for kt in range(NKT):
                nc.tensor.matmul(
                    pss[kt][:, :], lhsT=ktF2[:, kt * 128 : (kt + 1) * 128], rhs=qT[:, qs],
                    start=False, stop=True, skip_group_check=True,
                )
                pt = work.tile([128, NQ], BF16, tag="ptile")
                nc.scalar.activation(pt[:, :], pss[kt][:, :], Act.Exp)
                pts[kt] = pt
                # next preload right away: its PSUM-write drain hides under
                # the PV/inject matmuls instead of stalling the next score
                if kt + 1 < NKT:
                    preload(kt + 1)
                # running row-sum contributions off the PE: 3/4 gpsimd, 1/4 DVE
                if kt == 0:
                    nc.gpsimd.tensor_copy(laccg[:, :], pt[:, :])
                elif kt == 3:
                    nc.vector.tensor_copy(laccv[:, :], pt[:, :])
                elif kt % 4 == 3:
                    nc.vector.tensor_add(laccv[:, :], laccv[:, :], pt[:, :])
                else:
                    nc.gpsimd.tensor_add(laccg[:, :], laccg[:, :], pt[:, :])
                if inject:
                    inject.pop(0)()
                # PV for kt-2 keeps PE two tiles behind ACT (no stall)
                if kt >= 2:
                    pv(kt - 2)
            for kt in (NKT - 2, NKT - 1):
                pv(kt)
            while inject:
                inject.pop(0)()

            # l = colsum(laccg + laccv) via ones matmul; then 1/l per q
            laccs = stat.tile([128, NQ], F32R, tag="laccs")
            nc.vector.tensor_add(laccs[:, :], laccg[:, :], laccv[:, :])
            psl = pssm.tile([1, NQ], FP32, tag="sm")
            nc.tensor.matmul(psl[0:1, :], lhsT=ones_cr[:, :], rhs=laccs[:, :],
                             start=True, stop=True)
            lrow = stat.tile([1, NQ], FP32, tag="lrow")
            nc.scalar.copy(lrow[0:1, :], psl[0:1, :])
            otn = work.tile([128, NQ], FP32, tag="otn")
            nc.scalar.copy(otn[:, :], pso[0][:, :])
            nc.vector.tensor_add(otn[:, :], otn[:, :], pso[1][:, :])
            for qt in range(NQ // 128):
                plt = pssm.tile([128, 1], FP32, tag="sm")
                nc.tensor.transpose(plt[:, 0:1], lrow[0:1, qt * 128 : (qt + 1) * 128], ident[0:1, 0:1])
                linv = stat.tile([128, 1], FP32, tag="linv")
                nc.vector.reciprocal(linv[:, :], plt[:, 0:1])
                po = psS.tile([128, 128], FP32, tag="ps")
                nc.tensor.transpose(po[:, :], otn[:, qt * 128 : (qt + 1) * 128], ident[:, :])
                ot = outp.tile([128, 128], FP32)
                nc.scalar.activation(ot[:, :], po[:, :], Act.Identity, scale=linv[:, :])
                q0 = g * NQ + qt * 128
                nc.gpsimd.dma_start(out=out_sh[q0 : q0 + 128, :], in_=ot[:, :])

        # p1 blocks 0-3 up front: all half-A chunks first (their gather
        # lands earlier), then half-B + finalize; blocks 4-7 inject into g0
        parts03 = [p1_parts(b) for b in range(4)]
        for A, _, _ in parts03:
            for cl in A:
                cl()
        for _, B, fin in parts03:
            for cl in B:
                cl()
            fin()
        p2_group(0, inject=p1_closures(4) + p1_closures(5) + p1_closures(6) + p1_closures(7))
        p2_group(1)

    split_multi_waits(nc)
    return nc


def split_multi_waits(nc):
    """Hoist all-but-one sync wait off engine/DMA instructions into
    standalone EventSemaphore instructions.

    This toolchain's walrus build has a single wait-command slot per
    non-sequencer instruction; Tile emits multi-wait instructions assuming
    a newer codegen. Sequencer sync instructions (EventSemaphore, Drain)
    accept arbitrary waits, so semantics are preserved by hoisting.
    """
    import bass_rust

    exempt = {"InstEventSemaphore"}
    n_split = 0
    for f in nc.m.functions:
        for bb in f.blocks:
            out = []
            changed = False
            for ins in bb.instructions:
                si = ins.sync_info
                if (
                    si is not None
                    and len(si.on_wait) > 1
                    and type(ins).__name__ not in exempt
                    and ins.engine is not None
                ):
                    for j, w in enumerate(si.on_wait[:-1]):
                        ev = mybir.InstEventSemaphore(
                            name=f"{ins.name}-wsplit{j}", ins=[], outs=[]
                        )
                        ev.engine = ins.engine
                        ev.sync_info = bass_rust.SyncInfo(on_wait=[w], on_update=[])
                        out.append(ev)
                        n_split += 1
                    ins.sync_info = bass_rust.SyncInfo(
                        on_wait=[si.on_wait[-1]], on_update=list(si.on_update)
                    )
                    changed = True
                out.append(ins)
            if changed:
                bb.instructions = out
    return n_split


_PROGRAM = None


def _get_program():
    global _PROGRAM
    if _PROGRAM is None:
        _PROGRAM = build_program()
    return _PROGRAM


def kernel(x, W_Q, b_Q, W_K, b_K, W_V, b_V):
    x = np.ascontiguousarray(np.asarray(x, dtype=np.float32))
    args = {
        "W_Q": np.ascontiguousarray(np.asarray(W_Q, dtype=np.float32)),
        "b_Q": np.ascontiguousarray(np.asarray(b_Q, dtype=np.float32)),
        "W_K": np.ascontiguousarray(np.asarray(W_K, dtype=np.float32)),
        "b_K": np.ascontiguousarray(np.asarray(b_K, dtype=np.float32)),
        "W_V": np.ascontiguousarray(np.asarray(W_V, dtype=np.float32)),
        "b_V": np.ascontiguousarray(np.asarray(b_V, dtype=np.float32)),
    }
    nc = _get_program()
    in_maps = [dict(args, x_sh=x[c * SL : (c + 1) * SL]) for c in range(NC)]
    res = run_bass_kernel_spmd(nc, in_maps, list(range(NC)))
    return np.concatenate([res.results[c]["out_sh"] for c in range(NC)], axis=0)


# revision 29
# speedup vs baseline: 1.1072x; 1.0478x over previous
"""Single-head attention (S=8192, D=1024, d_k=128) on 8 TRN2 NeuronCores.

Strategy: sequence-parallel. Each core owns SL=1024 query rows. Per core:
  - transpose the x shard via PE transposes to get x^T tiles
  - project K^T (dual epilogue: f32r + bf16 copies straight from PSUM),
    V^T (PE-transposed to V natural, stored bf16), then Q^T with SCALE
    folded into the activation epilogue (f32r)
  - three AllGathers, ordered to pipeline the serial collective stream:
    bf16 K^T first (half the bytes -> the DVE-paced max phase starts
    early), then f32r K^T (feeds p2 scores), then bf16 V; gathers are
    issued right after their producers so transfers start as soon as the
    ~60us CC init barrier ends
  - max phase (p1), per 128-query block in natural orientation (bf16):
    Q^T block stationary, K^T moving -> S_nat chunks in PSUM; DVE
    reduce_max over the free dim straight from PSUM; combine partials,
    negate, PE-transpose into a [1, SL] row of -rowmax. p1 is DVE-paced
    (~700ns per 512-wide PSUM reduce), so blocks 0-3 run before group 0
    and blocks 4-7 are injected chunk-by-chunk into group 0's k-loop to
    keep the PE busy while the DVE chews.
  - attention phase (p2), per 512-query group, per 128-k tile:
    PSUM := -m via a K=1 bf16 ones x negmrow matmul (start=True) with the
    NEXT tile's preload emitted right after this tile's exp so its PSUM
    drain hides under other matmuls; S^T matmul (f32r, 1 cyc/row)
    accumulates on top (start=False) -> PSUM holds SCALE*S - m; ACT exps
    PSUM -> SBUF bf16 P^T tile; PV matmuls accumulate O^T across k into
    even/odd PSUM banks (alternating banks overlaps the accumulation
    drain); row sums l accumulate off-PE via gpsimd (3/4) and DVE (1/4)
    running adds over the P^T tiles
  - epilogue per group: l = ones^T @ (laccg + laccv) matmul, transpose,
    DVE reciprocal -> per-q-partition 1/l; O^T halves merged (ACT copy +
    DVE add), PE-transposed, scaled by 1/l in the ACT copy, DMA out.

Matmul dtypes: fp32 operands are produced as float32r (1 cycle/row for
free-dim >= 256 vs 4 for plain fp32; walrus requires producers to round
explicitly); P/V/preload matmuls run in bf16. The row max only needs to
be within ~+-80 of the true max (the softmax shift cancels exactly), so
bf16/f32r rounding on the max path is harmless; exp arguments stay
<= ~+16 so nothing overflows.

Hard-won toolchain constraints baked in here:
  - walrus allows ONE sync wait per Matmult: DMA-fed matmul operands get
    a tiny "absorber" ldweights after their DMA, and split_multi_waits
    hoists any remaining extra waits into EventSemaphore instructions
  - a lone start=False f32r matmul onto engine-written PSUM only
    accumulates its first 128-column row group (hence the matmul preload,
    not an ACT/DVE copy); DVE/ACT writes to PSUM can't replace it
  - GPSIMD cannot access PSUM at all
  - collective triggers block the issuing engine (gpsimd) while a prior
    collective is in flight; gathered-tensor loads stay on gpsimd since
    SP-issued loads showed a sporadic race against collective completion
  - matmul PSUM outputs must fit one 2KB bank -> 512 fp32 free-dim cap
"""

import math
import os
import sys
from contextlib import ExitStack

for _p in ("/opt/trn_rl_repo", os.path.expanduser("~/.axon_site/_ro/trn_rl_repo")):
    if os.path.isdir(_p) and _p not in sys.path:
        sys.path.insert(0, _p)

import numpy as np

import concourse.bass as bass
import concourse.mybir as mybir
import concourse.tile as tile
from concourse.bass_utils import run_bass_kernel_spmd
from concourse.masks import make_identity

S = 8192
D = 1024
DK = 128
NC = 8
SL = S // NC  # 1024 query rows per core
SCALE = 1.0 / math.sqrt(DK)
FP32 = mybir.dt.float32
F32R = mybir.dt.float32r
BF16 = mybir.dt.bfloat16
Act = mybir.ActivationFunctionType
Alu = mybir.AluOpType

NQ = 512          # queries per p2 group
NG = SL // NQ     # 4 groups
NB = SL // 128    # 8 query blocks for p1
NKT = S // 128    # 64 k tiles
KC = 512          # k chunk width for p1 (one PSUM bank)
NCH = S // KC     # 16 p1 chunks per block


def build_program() -> bass.Bass:
    nc = bass.Bass(num_devices=NC)

    x_sh = nc.declare_dram_parameter("x_sh", [SL, D], FP32, isOutput=False)
    w_q = nc.declare_dram_parameter("W_Q", [D, DK], FP32, isOutput=False)
    b_q = nc.declare_dram_parameter("b_Q", [1, DK], FP32, isOutput=False)
    w_k = nc.declare_dram_parameter("W_K", [D, DK], FP32, isOutput=False)
    b_k = nc.declare_dram_parameter("b_K", [1, DK], FP32, isOutput=False)
    w_v = nc.declare_dram_parameter("W_V", [D, DK], FP32, isOutput=False)
    b_v = nc.declare_dram_parameter("b_V", [1, DK], FP32, isOutput=False)
    out_sh = nc.declare_dram_parameter("out_sh", [SL, DK], FP32, isOutput=True)

    groups = [list(range(NC))]

    with tile.TileContext(nc) as tc, ExitStack() as ctx:
        dram = ctx.enter_context(tc.tile_pool(name="dram", bufs=1, space="DRAM"))
        khl_d = dram.tile([DK, SL], BF16)
        khg_d = dram.tile([NC * DK, SL], BF16, addr_space="Shared")
        ktl_d = dram.tile([DK, SL], F32R)
        ktg_d = dram.tile([NC * DK, SL], F32R, addr_space="Shared")
        vnl_d = dram.tile([SL, DK], BF16)
        vng_d = dram.tile([S, DK], BF16, addr_space="Shared")
        const = ctx.enter_context(tc.tile_pool(name="const", bufs=1))
        big = ctx.enter_context(tc.tile_pool(name="big", bufs=1))
        stat = ctx.enter_context(tc.tile_pool(name="stat", bufs=2))
        work = ctx.enter_context(tc.tile_pool(name="work", bufs=4))
        outp = ctx.enter_context(tc.tile_pool(name="outp", bufs=3))
        # PSUM budget (8 banks): psP1 2x[128,512] (also hosts proj matmuls)
        #   + psS 3x[128,256] (scores, 128x128 transposes) + psO 1 + psL 1
        #   + pssm 1 (tiny stat transposes)
        psP1 = ctx.enter_context(tc.tile_pool(name="psP1", bufs=2, space="PSUM"))
        psS = ctx.enter_context(tc.tile_pool(name="psS", bufs=3, space="PSUM"))
        psO = ctx.enter_context(tc.tile_pool(name="psO", bufs=1, space="PSUM"))
        pssm = ctx.enter_context(tc.tile_pool(name="pssm", bufs=1, space="PSUM"))

        def absorb(col_ap):
            """1-wait PE ldweights folding col_ap's producer sem into PE's clock.

            Bare InstLdweights has no output, so it builds no WAW chain; the
            bf16 bitcast sidesteps the fp32 standalone-ldweights restriction
            (the loaded garbage weights are never used -- every real matmul
            self-loads since ldw-opt is disabled).
            """
            if os.environ.get("LDW_OPT", "0") != "1":
                nc.tensor.ldweights(weights=col_ap.bitcast(BF16))

        ident = const.tile([128, 128], FP32)
        make_identity(nc, ident[:, :])
        absorb(ident[:, 0:1])
        ones_rf = const.tile([1, 128], FP32, tag="ones_rf")
        nc.gpsimd.memset(ones_rf[:, :], 1.0)
        ones_row = const.tile([1, 128], F32R, tag="ones_row")
        nc.scalar.copy(ones_row[0:1, :], ones_rf[0:1, :])
        absorb(ones_row[0:1, 0:1])
        ones_rb = const.tile([1, 128], BF16, tag="ones_rb")
        nc.scalar.copy(ones_rb[0:1, :], ones_rf[0:1, :])
        absorb(ones_rb[0:1, 0:1])
        ones_cf = const.tile([128, 1], FP32, tag="ones_cf")
        nc.gpsimd.memset(ones_cf[:, :], 1.0)
        ones_cr = const.tile([128, 1], F32R, tag="ones_cr")
        nc.scalar.copy(ones_cr[:, 0:1], ones_cf[:, 0:1])
        absorb(ones_cr[:, 0:1])

        def load_bias_T(b_dram, tag):
            t = const.tile([128, 1], FP32, tag=tag)
            nc.sync.dma_start(out=t[:, 0], in_=b_dram[0, :])
            return t

        bqT = load_bias_T(b_q, "bqT")
        bkT = load_bias_T(b_k, "bkT")
        bvT = load_bias_T(b_v, "bvT")
        # Q epilogue computes SCALE*ps + (SCALE*b)
        bqTs = const.tile([128, 1], FP32, tag="bqTs")
        nc.scalar.mul(bqTs[:, :], bqT[:, :], SCALE)

        proj_ctx = ExitStack()
        xpool = proj_ctx.enter_context(tc.tile_pool(name="xpool", bufs=1))
        xload = proj_ctx.enter_context(tc.tile_pool(name="xload", bufs=8))
        wpool = proj_ctx.enter_context(tc.tile_pool(name="wpool", bufs=3))

        def load_w(w_dram, tag):
            wt = wpool.tile([128, D // 128, DK], FP32, tag="wt")
            nc.gpsimd.dma_start(out=wt[:, :, :], in_=w_dram.rearrange("(t p) k -> p t k", p=128))
            absorb(wt[:, 0, 0:1])
            wtr = wpool.tile([128, D // 128, DK], F32R, tag="wtr")
            nc.scalar.copy(wtr[:, :, :], wt[:, :, :])
            return wtr

        wK = load_w(w_k, "wK")
        wV = load_w(w_v, "wV")
        wQ = load_w(w_q, "wQ")

        # x^T: [128 d-part, 8 d-tiles, SL seq]  (d = dt*128 + partition)
        xT = xpool.tile([128, D // 128, SL], F32R)

        def xt_half(h):
            for st in range(h * 4, h * 4 + 4):
                xn = xload.tile([128, D], FP32)
                nc.gpsimd.dma_start(out=xn[:, :], in_=x_sh[st * 128 : (st + 1) * 128, :])
                absorb(xn[:, 0:1])
                for dt in range(D // 128):
                    pt = psS.tile([128, 128], FP32, tag="ps")
                    nc.tensor.transpose(pt[:, :], xn[:, dt * 128 : (dt + 1) * 128], ident[:, :])
                    if dt % 2 == 0:
                        nc.scalar.copy(xT[:, dt, st * 128 : (st + 1) * 128], pt[:, :])
                    else:
                        nc.vector.tensor_copy(xT[:, dt, st * 128 : (st + 1) * 128], pt[:, :])

        def proj_half(wtr, outT, bT, h, scale=1.0, outB=None):
            """outT[:, h*512:(h+1)*512] = (scale*(x_h @ W) + scale*b)^T;
            outB (optional, emitted FIRST) gets a bf16 copy of the same."""
            ps = psP1.tile([128, 512], FP32, tag="p1")
            for dt in range(D // 128):
                nc.tensor.matmul(
                    ps[:, :],
                    lhsT=wtr[:, dt, :],
                    rhs=xT[:, dt, h * 512 : (h + 1) * 512],
                    start=(dt == 0),
                    stop=(dt == D // 128 - 1),
                )
            if outB is not None:
                nc.scalar.activation(outB[:, h * 512 : (h + 1) * 512], ps[:, :], Act.Identity, bias=bT[:, :], scale=scale)
            nc.scalar.activation(outT[:, h * 512 : (h + 1) * 512], ps[:, :], Act.Identity, bias=bT[:, :], scale=scale)

        # K first; a bf16 copy of K^T gathers FIRST (half the bytes) so the
        # DVE-paced max phase can start before the f32r K and V land
        ktl = big.tile([128, SL], F32R)
        khl = big.tile([128, SL], BF16, tag="khl")
        xt_half(0)
        proj_half(wK, ktl, bkT, 0, outB=khl)
        xt_half(1)
        proj_half(wK, ktl, bkT, 1, outB=khl)
        nc.gpsimd.dma_start(out=khl_d[:, :], in_=khl[:, :])
        nc.gpsimd.collective_compute(
            "AllGather", Alu.bypass, replica_groups=groups, ins=[khl_d[:, :]], outs=[khg_d[:, :]]
        )
        nc.gpsimd.dma_start(out=ktl_d[:, :], in_=ktl[:, :])
        nc.gpsimd.collective_compute(
            "AllGather", Alu.bypass, replica_groups=groups, ins=[ktl_d[:, :]], outs=[ktg_d[:, :]]
        )

        vtl = big.tile([128, SL], F32R)
        proj_half(wV, vtl, bvT, 0)
        proj_half(wV, vtl, bvT, 1)
        vnl = big.tile([128, SL // 128, DK], BF16)
        for st in range(SL // 128):
            pt = psS.tile([128, 128], FP32, tag="ps")
            nc.tensor.transpose(pt[:, :], vtl.bitcast(FP32)[:, st * 128 : (st + 1) * 128], ident[:, :])
            nc.scalar.copy(vnl[:, st, :], pt[:, :])
        nc.gpsimd.dma_start(out=vnl_d.rearrange("(t p) k -> p t k", p=128), in_=vnl[:, :, :])
        nc.gpsimd.collective_compute(
            "AllGather", Alu.bypass, replica_groups=groups, ins=[vnl_d[:, :]], outs=[vng_d[:, :]]
        )

        # Q (not gather-dependent); pre-scaled by SCALE
        qT = big.tile([128, SL], F32R)
        proj_half(wQ, qT, bqTs, 0, scale=SCALE)
        proj_half(wQ, qT, bqTs, 1, scale=SCALE)
        proj_ctx.close()

        # gathered K^T [128 dk, 8192 k] (fp32) and V natural (bf16)
        qTb = big.tile([128, SL], BF16, tag="qTb")
        nc.scalar.copy(qTb[:, :], qT[:, :])
        _emit_local_p1 = True
        ktFb = big.tile([128, NC, SL], BF16, tag="ktFb")
        nc.gpsimd.dma_start(out=ktFb[:, :, :], in_=khg_d.rearrange("(c p) s -> p c s", p=128))
        absorb(ktFb[:, 0, 0:1])
        ktFb2 = ktFb.rearrange("p c s -> p (c s)")
        ktF = big.tile([128, NC, SL], F32R)
        nc.gpsimd.dma_start(out=ktF[:, :, :], in_=ktg_d.rearrange("(c p) s -> p c s", p=128))
        absorb(ktF[:, 0, 0:1])
        ktF2 = ktF.rearrange("p c s -> p (c s)")
        vnF = big.tile([128, S // 128, DK], BF16)
        nc.gpsimd.dma_start(out=vnF[:, 0 : S // 256, :], in_=vng_d.rearrange("(t p) k -> p t k", p=128)[:, 0 : S // 256, :])
        absorb(vnF[:, 0, 0:1])
        nc.gpsimd.dma_start(out=vnF[:, S // 256 :, :], in_=vng_d.rearrange("(t p) k -> p t k", p=128)[:, S // 256 :, :])
        absorb(vnF[:, S // 256, 0:1])

        # -rowmax as a [1, SL] row, built per 128-q block by p1
        negmrow = stat.tile([1, SL], BF16, tag="negmrow", bufs=1)
        pmax_all = stat.tile([128, NB, NCH + 2], FP32, tag="pmax", bufs=1)

        def p1_local(b):
            """Max over the core's OWN k-shard (in SBUF pre-gather) for block
            b -> pmax cols 16,17. The gathered pass re-covers this shard
            redundantly; max is idempotent so that is harmless."""
            for lc in range(2):
                ps = psP1.tile([128, KC], FP32, tag="p1")
                nc.tensor.matmul(
                    ps[:, :],
                    lhsT=qTb[:, b * 128 : (b + 1) * 128],
                    rhs=khl[:, lc * KC : (lc + 1) * KC],
                    start=True, stop=True,
                )
                nc.vector.reduce_max(pmax_all[:, b, NCH + lc : NCH + lc + 1], ps[:, :], axis=mybir.AxisListType.X)

        def p1_parts(b):
            """p1 for block b as (A_chunks, B_chunks, finalize) closures."""

            def chunk(c):
                def emit():
                    ps = psP1.tile([128, KC], FP32, tag="p1")
                    nc.tensor.matmul(
                        ps[:, :],
                        lhsT=qTb[:, b * 128 : (b + 1) * 128],
                        rhs=ktFb2[:, c * KC : (c + 1) * KC],
                        start=True, stop=True,
                    )
                    nc.vector.reduce_max(pmax_all[:, b, c : c + 1], ps[:, :], axis=mybir.AxisListType.X)
                return emit

            def finalize():
                negm = stat.tile([128, 1], FP32, tag="negm")
                nc.vector.reduce_max(negm[:, :], pmax_all[:, b, :], axis=mybir.AxisListType.X, negate=True)
                pt = pssm.tile([1, 128], FP32, tag="sm")
                nc.tensor.transpose(pt[0:1, :], negm[:, :], ident[:, :])
                nc.vector.tensor_copy(negmrow[0:1, b * 128 : (b + 1) * 128], pt[0:1, :])

            A = [chunk(c) for c in range(0, NCH, 2)]
            B = [chunk(c) for c in range(1, NCH, 2)]
            return A, B, finalize

        def p1_closures(b):
            A, B, fin = p1_parts(b)
            return A + B + [fin]

        def p2_group(g, inject=()):
            """exp + PV + running l for query group g; `inject` closures are
            drained one per k-iteration (p1 work for later blocks)."""
            inject = list(inject)
            qs = slice(g * NQ, (g + 1) * NQ)
            pso_e = psO.tile([128, NQ], FP32, tag="oe")
            pso_o = psO.tile([128, NQ], FP32, tag="oo")
            pso = [pso_e, pso_o]
            laccg = stat.tile([128, NQ], FP32, tag="laccg", bufs=1)
            laccv = stat.tile([128, NQ], FP32, tag="laccv", bufs=1)
            pts = [None] * NKT
            pss = [None] * NKT

            def preload(kt):
                ps = psS.tile([128, NQ], FP32, tag="ps")
                nc.tensor.matmul(
                    ps[:, :], lhsT=ones_rb[0:1, :], rhs=negmrow[0:1, qs],
                    start=True, stop=False, skip_group_check=True,
                )
                pss[kt] = ps

            def pv(kt):
                nc.tensor.matmul(
                    pso[kt % 2][:, :], lhsT=vnF[:, kt, :], rhs=pts[kt][:, :],
                    start=(kt < 2), stop=(kt >= NKT - 2), skip_group_check=True,
                )

            preload(0)
            for kt in range(NKT):
                nc.tensor.matmul(
                    pss[kt][:, :], lhsT=ktF2[:, kt * 128 : (kt + 1) * 128], rhs=qT[:, qs],
                    start=False, stop=True, skip_group_check=True,
                )
                pt = work.tile([128, NQ], BF16, tag="ptile")
                nc.scalar.activation(pt[:, :], pss[kt][:, :], Act.Exp)
                pts[kt] = pt
                # next preload right away: its PSUM-write drain hides under
                # the PV/inject matmuls instead of stalling the next score
                if kt + 1 < NKT:
                    preload(kt + 1)
                # running row-sum contributions off the PE: 3/4 gpsimd, 1/4 DVE
                if kt == 0:
                    nc.gpsimd.tensor_copy(laccg[:, :], pt[:, :])
                elif kt == 3:
                    nc.vector.tensor_copy(laccv[:, :], pt[:, :])
                elif kt % 4 == 3:
                    nc.vector.tensor_add(laccv[:, :], laccv[:, :], pt[:, :])
                else:
                    nc.gpsimd.tensor_add(laccg[:, :], laccg[:, :], pt[:, :])
                if inject:
                    inject.pop(0)()
                # PV for kt-2 keeps PE two tiles behind ACT (no stall)
                if kt >= 2:
                    pv(kt - 2)
            for kt in (NKT - 2, NKT - 1):
                pv(kt)
            while inject:
                inject.pop(0)()

            # l = colsum(laccg + laccv) via ones matmul; then 1/l per q
            laccs = stat.tile([128, NQ], F32R, tag="laccs")
            nc.vector.tensor_add(laccs[:, :], laccg[:, :], laccv[:, :])
            psl = pssm.tile([1, NQ], FP32, tag="sm")
            nc.tensor.matmul(psl[0:1, :], lhsT=ones_cr[:, :], rhs=laccs[:, :],
                             start=True, stop=True)
            lrow = stat.tile([1, NQ], FP32, tag="lrow")
            nc.scalar.copy(lrow[0:1, :], psl[0:1, :])
            otn = work.tile([128, NQ], FP32, tag="otn")
            nc.scalar.copy(otn[:, :], pso[0][:, :])
            nc.vector.tensor_add(otn[:, :], otn[:, :], pso[1][:, :])
            for qt in range(NQ // 128):
                plt = pssm.tile([128, 1], FP32, tag="sm")
                nc.tensor.transpose(plt[:, 0:1], lrow[0:1, qt * 128 : (qt + 1) * 128], ident[0:1, 0:1])
                linv = stat.tile([128, 1], FP32, tag="linv")
                nc.vector.reciprocal(linv[:, :], plt[:, 0:1])
                po = psS.tile([128, 128], FP32, tag="ps")
                nc.tensor.transpose(po[:, :], otn[:, qt * 128 : (qt + 1) * 128], ident[:, :])
                ot = outp.tile([128, 128], FP32)
                nc.scalar.activation(ot[:, :], po[:, :], Act.Identity, scale=linv[:, :])
                q0 = g * NQ + qt * 128
                nc.gpsimd.dma_start(out=out_sh[q0 : q0 + 128, :], in_=ot[:, :])

        # local-shard p1 head start for every block (runs pre-gather)
        for b in range(NB):
            p1_local(b)
        # p1 blocks 0-3 up front: all half-A chunks first (their gather
        # lands earlier), then half-B + finalize; blocks 4-7 inject into g0
        parts03 = [p1_parts(b) for b in range(4)]
        for A, _, _ in parts03:
            for cl in A:
                cl()
        for _, B, fin in parts03:
            for cl in B:
                cl()
            fin()
        p2_group(0, inject=p1_closures(4) + p1_closures(5) + p1_closures(6) + p1_closures(7))
        p2_group(1)

    split_multi_waits(nc)
    return nc


def split_multi_waits(nc):
    """Hoist all-but-one sync wait off engine/DMA instructions into
    standalone EventSemaphore instructions.

    This toolchain's walrus build has a single wait-command slot per
    non-sequencer instruction; Tile emits multi-wait instructions assuming
    a newer codegen. Sequencer sync instructions (EventSemaphore, Drain)
    accept arbitrary waits, so semantics are preserved by hoisting.
    """
    import bass_rust

    exempt = {"InstEventSemaphore"}
    n_split = 0
    for f in nc.m.functions:
        for bb in f.blocks:
            out = []
            changed = False
            for ins in bb.instructions:
                si = ins.sync_info
                if (
                    si is not None
                    and len(si.on_wait) > 1
                    and type(ins).__name__ not in exempt
                    and ins.engine is not None
                ):
                    for j, w in enumerate(si.on_wait[:-1]):
                        ev = mybir.InstEventSemaphore(
                            name=f"{ins.name}-wsplit{j}", ins=[], outs=[]
                        )
                        ev.engine = ins.engine
                        ev.sync_info = bass_rust.SyncInfo(on_wait=[w], on_update=[])
                        out.append(ev)
                        n_split += 1
                    ins.sync_info = bass_rust.SyncInfo(
                        on_wait=[si.on_wait[-1]], on_update=list(si.on_update)
                    )
                    changed = True
                out.append(ins)
            if changed:
                bb.instructions = out
    return n_split


_PROGRAM = None


def _get_program():
    global _PROGRAM
    if _PROGRAM is None:
        _PROGRAM = build_program()
    return _PROGRAM


def kernel(x, W_Q, b_Q, W_K, b_K, W_V, b_V):
    x = np.ascontiguousarray(np.asarray(x, dtype=np.float32))
    args = {
        "W_Q": np.ascontiguousarray(np.asarray(W_Q, dtype=np.float32)),
        "b_Q": np.ascontiguousarray(np.asarray(b_Q, dtype=np.float32)),
        "W_K": np.ascontiguousarray(np.asarray(W_K, dtype=np.float32)),
        "b_K": np.ascontiguousarray(np.asarray(b_K, dtype=np.float32)),
        "W_V": np.ascontiguousarray(np.asarray(W_V, dtype=np.float32)),
        "b_V": np.ascontiguousarray(np.asarray(b_V, dtype=np.float32)),
    }
    nc = _get_program()
    in_maps = [dict(args, x_sh=x[c * SL : (c + 1) * SL]) for c in range(NC)]
    res = run_bass_kernel_spmd(nc, in_maps, list(range(NC)))
    return np.concatenate([res.results[c]["out_sh"] for c in range(NC)], axis=0)
